# revision 1
# baseline (speedup 1.0000x reference)
"""Child-Sum Tree-LSTM (reference.py nn_ChildSumTreeLSTM) on 8 Trainium2
NeuronCores via Bass/Tile, SPMD.

Strategy: everything transposed (features on SBUF partitions, nodes on the
free dimension). Each core owns a contiguous slice of every level (levels
5..8); since children of a node are contiguous, the leaves->level-5
recursion is fully core-local (no collectives). The top levels (4..0,
341 nodes) are finished on the host in numpy during the gather step.

Matmuls run in bf16 (fp32 PSUM accumulation); the forget-gate fx term is
folded into the f-gate matmul via a step-0 broadcast rhs and all biases
ride in the activation instructions. The leaf level is computed in SBUF
groups consumed immediately by level-7 chunks (h/c never touch DRAM).
Emission is software-pipelined for the in-order TensorEngine, and the
child-sum runs incrementally on GpSimd as child chunks complete.
"""
import sys
sys.path.insert(0, '/opt/trn_rl_repo')
import numpy as np
import ml_dtypes
import concourse.bacc as bacc
import concourse.mybir as mybir
from concourse.tile import TileContext
from concourse.alu_op_type import AluOpType

F32 = mybir.dt.float32
BF16 = mybir.dt.bfloat16
AFT = mybir.ActivationFunctionType
P = 128
NCORES = 8
BR = 4


def level_offs(D):
    return [(BR ** l - 1) // (BR - 1) for l in range(D + 1)]


def local_counts(D, cut):
    return {l: BR ** l // NCORES for l in range(cut, D + 1)}


def local_offs(D, cut):
    n = local_counts(D, cut)
    offs = {}
    acc = 0
    for l in range(cut, D + 1):
        offs[l] = acc
        acc += n[l]
    return offs, acc


def build_program(D, cut, chunk=512, c_dtype=BF16, hs_gpsimd=True):
    nloc = local_counts(D, cut)
    loff, total_rows = local_offs(D, cut)
    CDT = c_dtype

    nc = bacc.Bacc("TRN2", target_bir_lowering=False, debug=False,
                   num_devices=NCORES)
    xT = nc.dram_tensor("xT", [2, P, total_rows], BF16, kind="ExternalInput")
    wx = nc.dram_tensor("wx", [2, P, 1024], BF16, kind="ExternalInput")
    wh = nc.dram_tensor("wh", [2, P, 1024], BF16, kind="ExternalInput")
    bias = nc.dram_tensor("bias", [P, 8], F32, kind="ExternalInput")
    ncut = nloc[cut]
    out_h = nc.dram_tensor("out_h", [2, P, ncut], BF16, kind="ExternalOutput")
    out_c = nc.dram_tensor("out_c", [2, P, ncut], CDT, kind="ExternalOutput")

    with TileContext(nc) as tc:
        with tc.tile_pool(name="const", bufs=1) as constp, \
             tc.tile_pool(name="xin", bufs=2) as xin, \
             tc.tile_pool(name="state", bufs=1) as statep, \
             tc.tile_pool(name="leafg", bufs=3) as leafg, \
             tc.tile_pool(name="work", bufs=2) as work, \
             tc.tile_pool(name="psum", bufs=4, space="PSUM") as psum:

            wxt = constp.tile([P, 2, 1024], BF16)
            wht = constp.tile([P, 2, 1024], BF16)
            bt = constp.tile([P, 8], F32)
            nc.sync.dma_start(wxt[:], wx[:].rearrange("a p n -> p a n"))
            nc.sync.dma_start(wht[:], wh[:].rearrange("a p n -> p a n"))
            nc.sync.dma_start(bt[:], bias[:])

            def load_x(l, c0, S, tag="xt", bufs=2):
                t = xin.tile([P, 2, S], BF16, tag=tag, bufs=bufs, name=tag)
                src = xT[:, :, loff[l] + c0: loff[l] + c0 + S]
                nc.sync.dma_start(t[:], src.rearrange("a p n -> p a n"))
                return t

            def gate_tiles(S, pfx=""):
                it = work.tile([P, 2, S], BF16, tag=pfx + "i", name="it")
                ot = work.tile([P, 2, S], BF16, tag=pfx + "o", name="ot")
                ut = work.tile([P, 2, S], BF16, tag=pfx + "u", name="ut")
                return it, ot, ut

            def iou_matmuls(xt, S, hs=None, ptag="ps", pbufs=3):
                """Returns list of 6 psum tiles [P, S] (i0,i1,o0,o1,u0,u1)."""
                out = []
                for mt in range(6):
                    ps = psum.tile([P, S], F32, tag=ptag, bufs=pbufs, name="ps")
                    nc.tensor.matmul(ps[:], wxt[:, 0, mt * P:(mt + 1) * P],
                                     xt[:, 0, :], start=True, stop=False)
                    last = hs is None
                    nc.tensor.matmul(ps[:], wxt[:, 1, mt * P:(mt + 1) * P],
                                     xt[:, 1, :], start=False, stop=last)
                    if hs is not None:
                        nc.tensor.matmul(ps[:], wht[:, 0, mt * P:(mt + 1) * P],
                                         hs[:, 0, :], start=False, stop=False)
                        nc.tensor.matmul(ps[:], wht[:, 1, mt * P:(mt + 1) * P],
                                         hs[:, 1, :], start=False, stop=True)
                    out.append(ps)
                return out

            def gates_from_psums(iou, it, ot, ut, S):
                for ft in range(2):
                    nc.scalar.activation(it[:, ft, :], iou[ft][:], AFT.Sigmoid,
                                         bias=bt[:, ft:ft + 1])
                    nc.scalar.activation(ot[:, ft, :], iou[2 + ft][:], AFT.Sigmoid,
                                         bias=bt[:, 2 + ft:3 + ft])
                    nc.scalar.activation(ut[:, ft, :], iou[4 + ft][:], AFT.Tanh,
                                         bias=bt[:, 4 + ft:5 + ft])

            def leaf_chunk(xt, S, h_dst, c_dst):
                iou = iou_matmuls(xt, S, ptag="psl")
                it, ot, ut = gate_tiles(S, pfx="l")
                gates_from_psums(iou, it, ot, ut, S)
                # fused over both ftiles
                with nc.allow_low_precision(reason="bf16 by design"):
                    nc.vector.tensor_tensor(c_dst, it[:], ut[:], AluOpType.mult)
                    nc.scalar.activation(ut[:], c_dst, AFT.Tanh)
                    nc.vector.tensor_tensor(h_dst, ot[:], ut[:], AluOpType.mult)

            def internal_chunk(l, c0, S, ch_h, ch_c, h_dst, c_dst, hs):
                xt = load_x(l, c0, S)
                # forget gates first: fh@child_h + fx@x_parent (broadcast rhs)
                nch = BR * S
                fw = min(1024, nch)          # f-psum width (<=2 banks)
                ft_tile = work.tile([P, 2, nch], BF16, tag="f", name="ft_tile")
                for ftt in range(2):
                    for q in range(nch // fw):
                        psf = psum.tile([P, fw], F32, tag="psf", bufs=1,
                                        name="psf")
                        for half in range(fw // 512) or [0]:
                            lo = q * fw + half * 512
                            w_ = min(512, nch - lo)
                            dst = psf[:, half * 512: half * 512 + w_]
                            nc.tensor.matmul(
                                dst, wht[:, 0, (768 + ftt * P):(768 + (ftt + 1) * P)],
                                ch_h[:, 0, lo:lo + w_], start=True, stop=False)
                            nc.tensor.matmul(
                                dst, wht[:, 1, (768 + ftt * P):(768 + (ftt + 1) * P)],
                                ch_h[:, 1, lo:lo + w_], start=False, stop=False)
                            plo, pw = lo // BR, w_ // BR
                            for kt in range(2):
                                rhs = xt[:, kt, plo:plo + pw] \
                                    .rearrange("p (n b) -> p n b", b=1) \
                                    .broadcast_to([P, pw, BR])
                                nc.tensor.matmul(
                                    dst.rearrange("p (n b) -> p n b", b=BR),
                                    wxt[:, kt, (768 + ftt * P):(768 + (ftt + 1) * P)],
                                    rhs, start=False, stop=(kt == 1))
                        nc.scalar.activation(ft_tile[:, ftt, q * fw:(q + 1) * fw],
                                             psf[:], AFT.Sigmoid,
                                             bias=bt[:, 6 + ftt:7 + ftt])
                # f * c_child (in place), group-sum into fcs
                fcs = work.tile([P, 2, S], CDT, tag="fcs", name="fcs")
                with nc.allow_low_precision(reason="bf16 by design"):
                    nc.vector.tensor_tensor(ft_tile[:], ft_tile[:], ch_c,
                                            AluOpType.mult)
                    for ft in range(2):
                        nc.vector.tensor_reduce(
                            fcs[:, ft, :],
                            ft_tile[:, ft, :].rearrange("p (n b) -> p n b", b=BR),
                            mybir.AxisListType.X, AluOpType.add)
                iou = iou_matmuls(xt, S, hs)
                it, ot, ut = gate_tiles(S)
                gates_from_psums(iou, it, ot, ut, S)
                with nc.allow_low_precision(reason="bf16 by design"):
                    # c = i*u + fcs ; h = o * tanh(c)   (ftile-fused)
                    nc.vector.tensor_tensor(it[:], it[:], ut[:], AluOpType.mult)
                    nc.vector.tensor_tensor(c_dst, it[:], fcs[:], AluOpType.add)
                    nc.scalar.activation(ut[:], c_dst, AFT.Tanh)
                    nc.vector.tensor_tensor(h_dst, ot[:], ut[:], AluOpType.mult)

            # ---- persistent level tiles ----
            lt_h = {}
            lt_c = {}
            for l in range(cut, D):
                lt_h[l] = statep.tile([P, 2, nloc[l]], BF16, tag=f"h{l}",
                                      name=f"h{l}")
                lt_c[l] = statep.tile([P, 2, nloc[l]], CDT, tag=f"c{l}",
                                      name=f"c{l}")
            # child-sum accumulators, filled incrementally as child h completes
            hs_t = {}
            for l in range(cut, D):
                hs_t[l] = statep.tile([P, 2, nloc[l]], BF16, tag=f"hs{l}",
                                      name=f"hs{l}")

            def emit_hsum(lpar, ch_ap, c0p, Sp):
                """Sum 4-child groups of ch_ap ([P,2,4*Sp]) into
                hs_t[lpar][:, :, c0p:c0p+Sp]."""
                with nc.allow_low_precision(reason="bf16 by design"):
                    htmp = work.tile([P, 2, Sp, 2], BF16, tag="htmp", name="htmp")
                    for ft in range(2):
                        v = ch_ap[:, ft, :].rearrange("p (n b) -> p n b", b=BR)
                        nc.gpsimd.tensor_add(htmp[:, ft, :, :],
                                             v[:, :, 0:2], v[:, :, 2:4])
                        nc.gpsimd.tensor_add(hs_t[lpar][:, ft, c0p:c0p + Sp],
                                             htmp[:, ft, :, 0],
                                             htmp[:, ft, :, 1])

            # ---- leaf level fused with level D-1 ----
            lp = D - 1
            pc = min(chunk, nloc[lp])
            n_groups = nloc[lp] // pc
            leafc = pc * BR
            pending = None
            for g in range(n_groups):
                h8g = leafg.tile([P, 2, leafc], BF16, tag="h8g", name="h8g")
                c8g = leafg.tile([P, 2, leafc], CDT, tag="c8g", name="c8g")
                lsub = min(chunk, leafc)
                for s in range(leafc // lsub):
                    xt = load_x(D, g * leafc + s * lsub, lsub, tag="xleaf",
                                bufs=4)
                    hsl = h8g[:, :, s * lsub:(s + 1) * lsub]
                    leaf_chunk(xt, lsub, hsl,
                               c8g[:, :, s * lsub:(s + 1) * lsub])
                    emit_hsum(lp, hsl, g * pc + s * lsub // BR, lsub // BR)
                if pending is not None:
                    internal_chunk(*pending)
                pending = (lp, g * pc, pc, h8g[:], c8g[:],
                           lt_h[lp][:, :, g * pc:(g + 1) * pc],
                           lt_c[lp][:, :, g * pc:(g + 1) * pc],
                           hs_t[lp][:, :, g * pc:(g + 1) * pc])
            internal_chunk(*pending)
            # ---- levels D-2 .. cut ----
            for l in range(D - 2, cut - 1, -1):
                # child-sum for this level's parents from level l+1 h
                emit_hsum(l, lt_h[l + 1][:], 0, nloc[l])
                S = nloc[l]
                pcS = min(chunk, S)
                for c0 in range(0, S, pcS):
                    internal_chunk(l, c0, pcS,
                                   lt_h[l + 1][:, :, c0 * BR:(c0 + pcS) * BR],
                                   lt_c[l + 1][:, :, c0 * BR:(c0 + pcS) * BR],
                                   lt_h[l][:, :, c0:c0 + pcS],
                                   lt_c[l][:, :, c0:c0 + pcS],
                                   hs_t[l][:, :, c0:c0 + pcS])

            nc.sync.dma_start(out_h[:].rearrange("a p n -> p a n"), lt_h[cut][:])
            nc.sync.dma_start(out_c[:].rearrange("a p n -> p a n"), lt_c[cut][:])

    nc.compile()
    return nc


def shard_inputs(x, W_iou_x, b_iou_x, W_iou_h, b_iou_h, W_fx, b_fx, W_fh, b_fh,
                 D, cut):
    offs = level_offs(D)
    nloc = local_counts(D, cut)
    wx_cat = np.concatenate([W_iou_x, W_fx], axis=0)
    wh_cat = np.concatenate([W_iou_h, W_fh], axis=0)
    wx_d = np.ascontiguousarray(wx_cat.T).reshape(2, P, 1024).astype(ml_dtypes.bfloat16)
    wh_d = np.ascontiguousarray(wh_cat.T).reshape(2, P, 1024).astype(ml_dtypes.bfloat16)
    b_iou = (b_iou_x + b_iou_h).reshape(6, P).T
    b_f = (b_fx + b_fh).reshape(2, P).T
    bias = np.ascontiguousarray(
        np.concatenate([b_iou, b_f], axis=1)).astype(np.float32)
    in_maps = []
    for k in range(NCORES):
        rows = []
        for l in range(cut, D + 1):
            n = nloc[l]
            rows.append(x[offs[l] + k * n: offs[l] + (k + 1) * n])
        xl = np.concatenate(rows, axis=0)
        xTk = np.ascontiguousarray(xl.T).reshape(2, P, -1).astype(ml_dtypes.bfloat16)
        in_maps.append({"xT": xTk, "wx": wx_d, "wh": wh_d, "bias": bias})
    return in_maps


def finish_host(results, x, W_iou_x, b_iou_x, W_iou_h, b_iou_h,
                W_fx, b_fx, W_fh, b_fh, D, cut):
    ncut = BR ** cut
    npc = ncut // NCORES
    Hc = np.empty((ncut, 256), np.float32)
    Cc = np.empty((ncut, 256), np.float32)
    for k in range(NCORES):
        oh = results[k]["out_h"].astype(np.float32).reshape(256, npc)
        oc = results[k]["out_c"].astype(np.float32).reshape(256, npc)
        Hc[k * npc:(k + 1) * npc] = oh.T
        Cc[k * npc:(k + 1) * npc] = oc.T
    sig = lambda v: 1.0 / (1.0 + np.exp(-v))
    h_next, c_next = Hc, Cc
    for l in range(cut - 1, -1, -1):
        n, off = BR ** l, (BR ** l - 1) // 3
        xl = x[off:off + n]
        child_h = h_next.reshape(n, BR, 256)
        child_c = c_next.reshape(n, BR, 256)
        chs = child_h.sum(axis=1)
        iou = xl @ W_iou_x.T + b_iou_x + chs @ W_iou_h.T + b_iou_h
        i, o, u = np.split(iou, 3, axis=1)
        i, o, u = sig(i), sig(o), np.tanh(u)
        f = sig(child_h @ W_fh.T + b_fh + (xl @ W_fx.T + b_fx)[:, None, :])
        c = i * u + (f * child_c).sum(axis=1)
        h = o * np.tanh(c)
        h_next, c_next = h, c
    return c_next.astype(np.float32), h_next.astype(np.float32)


# ---------------- public API ----------------

_D = 8
_CUT = 5
_CACHE = {}


def _get_program():
    if "nc" not in _CACHE:
        _CACHE["nc"] = build_program(_D, _CUT)
    return _CACHE["nc"]


def kernel(x, W_iou_x, b_iou_x, W_iou_h, b_iou_h, W_fx, b_fx, W_fh, b_fh):
    from concourse import bass_utils
    x = np.asarray(x, dtype=np.float32)
    args = [np.asarray(a, dtype=np.float32) for a in
            (W_iou_x, b_iou_x, W_iou_h, b_iou_h, W_fx, b_fx, W_fh, b_fh)]
    nc = _get_program()
    in_maps = shard_inputs(x, *args, _D, _CUT)
    res = bass_utils.run_bass_kernel_spmd(nc, in_maps,
                                          core_ids=list(range(NCORES)))
    c, h = finish_host(res.results, x, *args, _D, _CUT)
    return c, h



# revision 7
# speedup vs baseline: 1.2599x; 1.2599x over previous
"""Child-Sum Tree-LSTM (reference.py nn_ChildSumTreeLSTM) on 8 Trainium2
NeuronCores via Bass/Tile, SPMD.

Strategy: everything transposed (features on SBUF partitions, nodes on the
free dimension). Each core owns a contiguous slice of levels 6..8; since
children of a node are contiguous, the leaves->level-6 recursion is fully
core-local (no collectives). Levels 5..0 (1365 nodes) are finished on the
host in numpy during the gather step.

v2 vs v1: ACT-engine-centred redesign. The activation engine is the real
bottleneck (1 elem/cycle/lane + ~352-cycle fixed cost per ACTIVATE), so
gates are produced from [P, 2048] four-bank PSUM tiles and activated in
N=2048 calls (N=4096 for the tanh(c) that needs no bias). fp16 replaces
bf16 at identical engine throughput for ~8x less quantization noise.
Leaf groups are software-pipelined one group ahead of the level-7
forget-gate matmuls so the TensorEngine never stalls on the activation
chain it just fed (keeps the PE HAM clock at 2.4 GHz), and level-7 iou /
level-6 f work is interleaved in halves to shorten the serial tail.
"""
import sys
sys.path.insert(0, '/opt/trn_rl_repo')
import numpy as np
import concourse.bacc as bacc
import concourse.mybir as mybir
from concourse.tile import TileContext
from concourse.alu_op_type import AluOpType

F32 = mybir.dt.float32
F16 = mybir.dt.float16
AFT = mybir.ActivationFunctionType
P = 128
NCORES = 8
BR = 4
D = 8
CUT = 6

# per-core local node counts / row offsets inside the packed xT tensor
NLOC = {6: 512, 7: 2048, 8: 8192}
LOFF = {6: 0, 7: 512, 8: 2560}
TOTAL_ROWS = 10752
GL = 2048                     # leaf group width (nodes)
NG = NLOC[8] // GL            # 4 leaf groups

# gate order of emission: (wx block, act fn, name) ; wx free layout is
# [i(256) | o(256) | u(256) | f(256)], bias cols [i0,i1,o0,o1,u0,u1,f0,f1]
GATES = ((0, AFT.Sigmoid, "i"), (2, AFT.Tanh, "u"), (1, AFT.Sigmoid, "o"))


def build_program():
    nc = bacc.Bacc("TRN2", target_bir_lowering=False, debug=False,
                   num_devices=NCORES)
    xT = nc.dram_tensor("xT", [2, P, TOTAL_ROWS], F16, kind="ExternalInput")
    wx = nc.dram_tensor("wx", [2, P, 1024], F16, kind="ExternalInput")
    wh = nc.dram_tensor("wh", [2, P, 1024], F16, kind="ExternalInput")
    bias = nc.dram_tensor("bias", [P, 8], F32, kind="ExternalInput")
    out_h = nc.dram_tensor("out_h", [2, P, NLOC[6]], F16, kind="ExternalOutput")
    out_c = nc.dram_tensor("out_c", [2, P, NLOC[6]], F16, kind="ExternalOutput")

    with TileContext(nc) as tc:
        with tc.tile_pool(name="const", bufs=1) as constp, \
             tc.tile_pool(name="xin", bufs=2) as xin, \
             tc.tile_pool(name="state", bufs=1) as statep, \
             tc.tile_pool(name="leafg", bufs=2) as leafg, \
             tc.tile_pool(name="work", bufs=2) as work, \
             tc.tile_pool(name="psum", bufs=2, space="PSUM") as psum:

            wxt = constp.tile([P, 2, 1024], F16)
            bt = constp.tile([P, 8], F32)
            wht = constp.tile([P, 2, 1024], F16)
            nc.sync.dma_start(wxt[:], wx[:].rearrange("a p n -> p a n"))
            nc.sync.dma_start(bt[:], bias[:])

            def load_x(l, c0, S, tag, bufs=2):
                t = xin.tile([P, 2, S], F16, tag=tag, bufs=bufs, name=tag)
                src = xT[:, :, LOFF[l] + c0: LOFF[l] + c0 + S]
                nc.sync.dma_start(t[:], src.rearrange("a p n -> p a n"))
                return t

            # first leaf x load goes out before the big weight/x DMAs so the
            # PE can start as early as possible
            xt_g = [load_x(8, 0, GL, tag="xleaf", bufs=2)]
            nc.sync.dma_start(wht[:], wh[:].rearrange("a p n -> p a n"))
            x7 = load_x(7, 0, NLOC[7], tag="x7", bufs=1)
            x6 = load_x(6, 0, NLOC[6], tag="x6", bufs=1)

            # persistent level state
            hs7 = statep.tile([P, 2, NLOC[7]], F16, name="hs7")
            fcs7 = statep.tile([P, 2, NLOC[7]], F16, name="fcs7")
            h7 = statep.tile([P, 2, NLOC[7]], F16, name="h7")
            c7 = statep.tile([P, 2, NLOC[7]], F16, name="c7")
            hs6 = statep.tile([P, 2, NLOC[6]], F16, name="hs6")
            fcs6 = statep.tile([P, 2, NLOC[6]], F16, name="fcs6")
            h6 = statep.tile([P, 2, NLOC[6]], F16, name="h6")
            c6 = statep.tile([P, 2, NLOC[6]], F16, name="c6")

            def iou_psum(xt, S, gi, ft, hs=None):
                """[P, S] psum for gate-block gi, feature-tile ft."""
                ps = psum.tile([P, 2048], F32, tag="PS", bufs=2, name="ps")
                sl = slice((gi * 2 + ft) * P, (gi * 2 + ft + 1) * P)
                for q in range(0, S, 512):
                    w = min(512, S - q)
                    dst = ps[:, q:q + w]
                    nc.tensor.matmul(dst, wxt[:, 0, sl], xt[:, 0, q:q + w],
                                     start=True, stop=False)
                    nc.tensor.matmul(dst, wxt[:, 1, sl], xt[:, 1, q:q + w],
                                     start=False, stop=hs is None)
                    if hs is not None:
                        nc.tensor.matmul(dst, wht[:, 0, sl], hs[:, 0, q:q + w],
                                         start=False, stop=False)
                        nc.tensor.matmul(dst, wht[:, 1, sl], hs[:, 1, q:q + w],
                                         start=False, stop=True)
                return ps

            def f_psum(ch_h, xp, S, ft):
                """[P, 4*S] forget-gate pre-activation psum for S parents:
                W_fh @ child_h + (W_fx @ x_parent) broadcast over children."""
                nch = BR * S
                ps = psum.tile([P, 2048], F32, tag="PS", bufs=2, name="psf")
                sl = slice(768 + ft * P, 768 + (ft + 1) * P)
                for q in range(0, nch, 512):
                    w = min(512, nch - q)
                    dst = ps[:, q:q + w]
                    nc.tensor.matmul(dst, wht[:, 0, sl], ch_h[:, 0, q:q + w],
                                     start=True, stop=False)
                    nc.tensor.matmul(dst, wht[:, 1, sl], ch_h[:, 1, q:q + w],
                                     start=False, stop=False)
                    plo, pw = q // BR, w // BR
                    for kt in range(2):
                        rhs = xp[:, kt, plo:plo + pw] \
                            .rearrange("p (n b) -> p n b", b=1) \
                            .broadcast_to([P, pw, BR])
                        nc.tensor.matmul(
                            dst.rearrange("p (n b) -> p n b", b=BR),
                            wxt[:, kt, sl], rhs, start=False, stop=(kt == 1))
                return ps

            def gates_block(xt, S, hs=None):
                """iou gates for S nodes -> (it, ut, ot) [P, 2, S] fp16.
                Gate tiles are shared across levels (bufs=1): the DVE combine
                consumes them long before the ACT queue wraps around."""
                tiles = {}
                for gi, fn, nm in GATES:
                    gt = work.tile([P, 2, 2048], F16, tag="g" + nm, bufs=1,
                                   name="g" + nm)
                    for ft in range(2):
                        ps = iou_psum(xt, S, gi, ft, hs)
                        nc.scalar.activation(gt[:, ft, :S], ps[:, :S], fn,
                                             bias=bt[:, gi * 2 + ft:gi * 2 + ft + 1])
                    tiles[nm] = gt[:, :, :S]
                return tiles["i"], tiles["u"], tiles["o"]

            def combine(it, ut, ot, c_dst, h_dst, fcs=None):
                """c = i*u (+ fcs); h = o*tanh(c). tanh reuses ut storage."""
                with nc.allow_low_precision(reason="fp16 by design"):
                    nc.vector.tensor_tensor(c_dst, it, ut, AluOpType.mult)
                    if fcs is not None:
                        nc.vector.tensor_tensor(c_dst, c_dst, fcs, AluOpType.add)
                    nc.scalar.activation(ut, c_dst, AFT.Tanh)
                    nc.vector.tensor_tensor(h_dst, ot, ut, AluOpType.mult)

            def emit_hsum(ch_h, dst, Sp):
                """Sum 4-child groups of ch_h [P,2,4*Sp] into dst [P,2,Sp]."""
                with nc.allow_low_precision(reason="fp16 by design"):
                    htmp = work.tile([P, 2, 512, 2], F16, tag="htmp", bufs=2,
                                     name="htmp")
                    for ft in range(2):
                        v = ch_h[:, ft, :].rearrange("p (n b) -> p n b", b=BR)
                        nc.gpsimd.tensor_add(htmp[:, ft, :Sp, :],
                                             v[:, :, 0:2], v[:, :, 2:4])
                        nc.gpsimd.tensor_add(dst[:, ft, :],
                                             htmp[:, ft, :Sp, 0],
                                             htmp[:, ft, :Sp, 1])

            def emit_fprod(f_sb, ch_c, dst, Sp):
                """dst[P,2,Sp] = sum_children sigmoid(f) * child_c."""
                with nc.allow_low_precision(reason="fp16 by design"):
                    nc.vector.tensor_tensor(f_sb[:], f_sb[:], ch_c,
                                            AluOpType.mult)
                    for ft in range(2):
                        nc.vector.tensor_reduce(
                            dst[:, ft, :],
                            f_sb[:, ft, :].rearrange("p (n b) -> p n b", b=BR),
                            mybir.AxisListType.X, AluOpType.add)

            # ---------------- leaf phase, pipelined with level-7 f ----------
            leaf_hc = [None] * NG

            def emit_leaf(g):
                if g + 1 < NG:
                    xt_g.append(load_x(8, (g + 1) * GL, GL, tag="xleaf"))
                xt = xt_g[g]
                it, ut, ot = gates_block(xt, GL)
                h8 = leafg.tile([P, 2, GL], F16, tag="h8", bufs=2, name="h8")
                c8 = leafg.tile([P, 2, GL], F16, tag="c8", bufs=2, name="c8")
                combine(it, ut, ot, c8[:], h8[:])
                emit_hsum(h8[:], hs7[:, :, g * 512:(g + 1) * 512], 512)
                leaf_hc[g] = (h8, c8)

            def emit_f7(g):
                h8, c8 = leaf_hc[g]
                f_sb = work.tile([P, 2, GL], F16, tag="f7", bufs=2, name="f7")
                for ft in range(2):
                    ps = f_psum(h8[:], x7[:, :, g * 512:(g + 1) * 512], 512, ft)
                    nc.scalar.activation(f_sb[:, ft, :], ps[:], AFT.Sigmoid,
                                         bias=bt[:, 6 + ft:7 + ft])
                emit_fprod(f_sb, c8[:], fcs7[:, :, g * 512:(g + 1) * 512], 512)

            emit_leaf(0)
            emit_leaf(1)
            emit_f7(0)
            emit_leaf(2)
            emit_f7(1)
            emit_leaf(3)
            emit_f7(2)

            # ---------------- level 7 iou in halves; level 6 f interleaved --
            HL = NLOC[7] // 2   # 1024
            def emit_iou7(h):
                s = slice(h * HL, (h + 1) * HL)
                it, ut, ot = gates_block(x7[:, :, s], HL, hs=hs7[:, :, s])
                combine(it, ut, ot, c7[:, :, s], h7[:, :, s],
                        fcs=fcs7[:, :, s])
                emit_hsum(h7[:, :, s], hs6[:, :, h * 256:(h + 1) * 256], 256)

            def emit_f6(h):
                f_sb = work.tile([P, 2, HL], F16, tag="f6", bufs=2, name="f6")
                s = slice(h * HL, (h + 1) * HL)
                for ft in range(2):
                    ps = f_psum(h7[:, :, s], x6[:, :, h * 256:(h + 1) * 256],
                                256, ft)
                    nc.scalar.activation(f_sb[:, ft, :], ps[:, :HL], AFT.Sigmoid,
                                         bias=bt[:, 6 + ft:7 + ft])
                emit_fprod(f_sb, c7[:, :, s],
                           fcs6[:, :, h * 256:(h + 1) * 256], 256)

            emit_iou7(0)
            emit_f7(3)
            emit_iou7(1)
            emit_f6(0)
            emit_f6(1)

            # ---------------- level 6 ----------------
            it, ut, ot = gates_block(x6[:], NLOC[6], hs=hs6[:])
            combine(it, ut, ot, c6[:], h6[:], fcs=fcs6[:])

            nc.sync.dma_start(out_h[:].rearrange("a p n -> p a n"), h6[:])
            nc.sync.dma_start(out_c[:].rearrange("a p n -> p a n"), c6[:])

    nc.compile()
    return nc


def level_offs():
    return [(BR ** l - 1) // (BR - 1) for l in range(D + 1)]


def shard_inputs(x, W_iou_x, b_iou_x, W_iou_h, b_iou_h, W_fx, b_fx, W_fh, b_fh,
                 *_ignored):
    offs = level_offs()
    wx_cat = np.concatenate([W_iou_x, W_fx], axis=0)
    wh_cat = np.concatenate([W_iou_h, W_fh], axis=0)
    wx_d = np.ascontiguousarray(wx_cat.T).reshape(2, P, 1024).astype(np.float16)
    wh_d = np.ascontiguousarray(wh_cat.T).reshape(2, P, 1024).astype(np.float16)
    b_iou = (b_iou_x + b_iou_h).reshape(6, P).T
    b_f = (b_fx + b_fh).reshape(2, P).T
    bias = np.ascontiguousarray(
        np.concatenate([b_iou, b_f], axis=1)).astype(np.float32)
    in_maps = []
    for k in range(NCORES):
        rows = []
        for l in range(CUT, D + 1):
            n = NLOC[l]
            rows.append(x[offs[l] + k * n: offs[l] + (k + 1) * n])
        xl = np.concatenate(rows, axis=0)
        xTk = np.ascontiguousarray(xl.T).reshape(2, P, -1).astype(np.float16)
        in_maps.append({"xT": xTk, "wx": wx_d, "wh": wh_d, "bias": bias})
    return in_maps


def finish_host(results, x, W_iou_x, b_iou_x, W_iou_h, b_iou_h,
                W_fx, b_fx, W_fh, b_fh, *_ignored):
    ncut = BR ** CUT
    npc = ncut // NCORES
    Hc = np.empty((ncut, 256), np.float32)
    Cc = np.empty((ncut, 256), np.float32)
    for k in range(NCORES):
        oh = results[k]["out_h"].astype(np.float32).reshape(256, npc)
        oc = results[k]["out_c"].astype(np.float32).reshape(256, npc)
        Hc[k * npc:(k + 1) * npc] = oh.T
        Cc[k * npc:(k + 1) * npc] = oc.T
    sig = lambda v: 1.0 / (1.0 + np.exp(-v))
    h_next, c_next = Hc, Cc
    for l in range(CUT - 1, -1, -1):
        n, off = BR ** l, (BR ** l - 1) // 3
        xl = x[off:off + n]
        child_h = h_next.reshape(n, BR, 256)
        child_c = c_next.reshape(n, BR, 256)
        chs = child_h.sum(axis=1)
        iou = xl @ W_iou_x.T + b_iou_x + chs @ W_iou_h.T + b_iou_h
        i, o, u = np.split(iou, 3, axis=1)
        i, o, u = sig(i), sig(o), np.tanh(u)
        f = sig(child_h @ W_fh.T + b_fh + (xl @ W_fx.T + b_fx)[:, None, :])
        c = i * u + (f * child_c).sum(axis=1)
        h = o * np.tanh(c)
        h_next, c_next = h, c
    return c_next.astype(np.float32), h_next.astype(np.float32)


# ---------------- public API ----------------

_D = D
_CUT = CUT
_CACHE = {}


def _get_program():
    if "nc" not in _CACHE:
        _CACHE["nc"] = build_program()
    return _CACHE["nc"]


def kernel(x, W_iou_x, b_iou_x, W_iou_h, b_iou_h, W_fx, b_fx, W_fh, b_fh):
    from concourse import bass_utils
    x = np.asarray(x, dtype=np.float32)
    args = [np.asarray(a, dtype=np.float32) for a in
            (W_iou_x, b_iou_x, W_iou_h, b_iou_h, W_fx, b_fx, W_fh, b_fh)]
    nc = _get_program()
    in_maps = shard_inputs(x, *args)
    res = bass_utils.run_bass_kernel_spmd(nc, in_maps,
                                          core_ids=list(range(NCORES)))
    c, h = finish_host(res.results, x, *args)
    return c, h


# revision 9
# speedup vs baseline: 1.3366x; 1.0609x over previous
"""Child-Sum Tree-LSTM (reference.py nn_ChildSumTreeLSTM) on 8 Trainium2
NeuronCores via Bass/Tile, SPMD.

Strategy: everything transposed (features on SBUF partitions, nodes on the
free dimension). Each core owns a contiguous slice of levels 7..8; since
children of a node are contiguous, the leaves->level-7 recursion is fully
core-local (no collectives). Levels 6..0 (5461 nodes) are finished on the
host in numpy during the gather step.

The activation engine is the binding resource (1 elem/cycle/lane plus a
~352-cycle fixed cost per ACTIVATE), so gates are produced from [P, 2048]
four-bank PSUM tiles and activated in N=2048 calls (N=4096 for the tanh(c)
that needs no bias). fp16 replaces bf16 at identical engine throughput for
~8x less quantization noise. Leaf groups are software-pipelined one group
ahead of the level-7 forget-gate matmuls so the TensorEngine never stalls
on the activation chain it just fed (keeps the PE HAM clock at 2.4 GHz).
Level-7 iou runs in halves with per-half output DMA so the final DMA
overlaps compute. Weights/x for later phases load on the Vector engine's
DMA queue in parallel with the Sync queue's leaf x stream.
"""
import sys
sys.path.insert(0, '/opt/trn_rl_repo')
import numpy as np
import concourse.bacc as bacc
import concourse.mybir as mybir
from concourse.tile import TileContext
from concourse.alu_op_type import AluOpType

F32 = mybir.dt.float32
F16 = mybir.dt.float16
AFT = mybir.ActivationFunctionType
P = 128
NCORES = 8
BR = 4
D = 8
CUT = 7

NLOC = {7: 2048, 8: 8192}
LOFF = {7: 0, 8: 2048}
TOTAL_ROWS = 10240
GL = 2048                     # leaf group width (nodes)
NG = NLOC[8] // GL            # 4 leaf groups

# gate emission order: (wx block index, act fn, name); wx free layout is
# [i(256) | o(256) | u(256) | f(256)], bias cols [i0,i1,o0,o1,u0,u1,f0,f1]
GATES = ((0, AFT.Sigmoid, "i"), (2, AFT.Tanh, "u"), (1, AFT.Sigmoid, "o"))


def build_program():
    nc = bacc.Bacc("TRN2", target_bir_lowering=False, debug=False,
                   num_devices=NCORES)
    xT = nc.dram_tensor("xT", [2, P, TOTAL_ROWS], F16, kind="ExternalInput")
    wx = nc.dram_tensor("wx", [2, P, 1024], F16, kind="ExternalInput")
    wh = nc.dram_tensor("wh", [2, P, 1024], F16, kind="ExternalInput")
    bias = nc.dram_tensor("bias", [P, 8], F32, kind="ExternalInput")
    out_h = nc.dram_tensor("out_h", [2, P, NLOC[7]], F16, kind="ExternalOutput")
    out_c = nc.dram_tensor("out_c", [2, P, NLOC[7]], F16, kind="ExternalOutput")

    with TileContext(nc) as tc:
        with tc.tile_pool(name="const", bufs=1) as constp, \
             tc.tile_pool(name="xin", bufs=2) as xin, \
             tc.tile_pool(name="state", bufs=1) as statep, \
             tc.tile_pool(name="leafg", bufs=2) as leafg, \
             tc.tile_pool(name="work", bufs=2) as work, \
             tc.tile_pool(name="psum", bufs=2, space="PSUM") as psum:

            wxt = constp.tile([P, 2, 1024], F16)
            bt = constp.tile([P, 8], F32)
            wht = constp.tile([P, 2, 1024], F16)
            nc.sync.dma_start(wxt[:], wx[:].rearrange("a p n -> p a n"))
            nc.sync.dma_start(bt[:], bias[:])

            def load_x(l, c0, S, tag, bufs=2, eng=None):
                t = xin.tile([P, 2, S], F16, tag=tag, bufs=bufs, name=tag)
                src = xT[:, :, LOFF[l] + c0: LOFF[l] + c0 + S]
                (eng or nc.sync).dma_start(t[:], src.rearrange("a p n -> p a n"))
                return t

            # first leaf x load on the sync queue; later-phase loads go out
            # on the vector queue in parallel
            xt_g = [load_x(8, 0, GL, tag="xleaf", bufs=2)]
            nc.scalar.dma_start(wht[:], wh[:].rearrange("a p n -> p a n"))
            x7 = load_x(7, 0, NLOC[7], tag="x7", bufs=1, eng=nc.scalar)

            # persistent level-7 state
            hs7 = statep.tile([P, 2, NLOC[7]], F16, name="hs7")
            fcs7 = statep.tile([P, 2, NLOC[7]], F16, name="fcs7")
            h7 = statep.tile([P, 2, NLOC[7]], F16, name="h7")
            c7 = statep.tile([P, 2, NLOC[7]], F16, name="c7")

            def iou_psum(xt, S, gi, ft, hs=None):
                """[P, S] psum for gate-block gi, feature-tile ft."""
                ps = psum.tile([P, 2048], F32, tag="PS", bufs=2, name="ps")
                sl = slice((gi * 2 + ft) * P, (gi * 2 + ft + 1) * P)
                for q in range(0, S, 512):
                    w = min(512, S - q)
                    dst = ps[:, q:q + w]
                    nc.tensor.matmul(dst, wxt[:, 0, sl], xt[:, 0, q:q + w],
                                     start=True, stop=False)
                    nc.tensor.matmul(dst, wxt[:, 1, sl], xt[:, 1, q:q + w],
                                     start=False, stop=hs is None)
                    if hs is not None:
                        nc.tensor.matmul(dst, wht[:, 0, sl], hs[:, 0, q:q + w],
                                         start=False, stop=False)
                        nc.tensor.matmul(dst, wht[:, 1, sl], hs[:, 1, q:q + w],
                                         start=False, stop=True)
                return ps

            def f_psum(ch_h, xp, S, ft):
                """[P, 4*S] forget-gate pre-activation psum for S parents:
                W_fh @ child_h + (W_fx @ x_parent) broadcast over children."""
                nch = BR * S
                ps = psum.tile([P, 2048], F32, tag="PS", bufs=2, name="psf")
                sl = slice(768 + ft * P, 768 + (ft + 1) * P)
                for q in range(0, nch, 512):
                    w = min(512, nch - q)
                    dst = ps[:, q:q + w]
                    nc.tensor.matmul(dst, wht[:, 0, sl], ch_h[:, 0, q:q + w],
                                     start=True, stop=False)
                    nc.tensor.matmul(dst, wht[:, 1, sl], ch_h[:, 1, q:q + w],
                                     start=False, stop=False)
                    plo, pw = q // BR, w // BR
                    for kt in range(2):
                        rhs = xp[:, kt, plo:plo + pw] \
                            .rearrange("p (n b) -> p n b", b=1) \
                            .broadcast_to([P, pw, BR])
                        nc.tensor.matmul(
                            dst.rearrange("p (n b) -> p n b", b=BR),
                            wxt[:, kt, sl], rhs, start=False, stop=(kt == 1))
                return ps

            def gates_block(xt, S, hs=None):
                """iou gates for S nodes -> (it, ut, ot) [P, 2, S] fp16.
                Gate tiles are shared across phases (bufs=1): the DVE combine
                consumes them long before the ACT queue wraps around."""
                tiles = {}
                for gi, fn, nm in GATES:
                    gt = work.tile([P, 2, 2048], F16, tag="g" + nm, bufs=1,
                                   name="g" + nm)
                    for ft in range(2):
                        ps = iou_psum(xt, S, gi, ft, hs)
                        nc.scalar.activation(gt[:, ft, :S], ps[:, :S], fn,
                                             bias=bt[:, gi * 2 + ft:gi * 2 + ft + 1])
                    tiles[nm] = gt[:, :, :S]
                return tiles["i"], tiles["u"], tiles["o"]

            def combine(it, ut, ot, c_dst, h_dst, fcs=None):
                """c = i*u (+ fcs); h = o*tanh(c). tanh reuses ut storage."""
                with nc.allow_low_precision(reason="fp16 by design"):
                    nc.vector.tensor_tensor(c_dst, it, ut, AluOpType.mult)
                    if fcs is not None:
                        nc.vector.tensor_tensor(c_dst, c_dst, fcs, AluOpType.add)
                    nc.scalar.activation(ut, c_dst, AFT.Tanh)
                    nc.vector.tensor_tensor(h_dst, ot, ut, AluOpType.mult)

            def emit_hsum(ch_h, dst, Sp):
                """Sum 4-child groups of ch_h [P,2,4*Sp] into dst [P,2,Sp]."""
                with nc.allow_low_precision(reason="fp16 by design"):
                    htmp = work.tile([P, 2, 512, 2], F16, tag="htmp", bufs=2,
                                     name="htmp")
                    for ft in range(2):
                        v = ch_h[:, ft, :].rearrange("p (n b) -> p n b", b=BR)
                        nc.gpsimd.tensor_add(htmp[:, ft, :Sp, :],
                                             v[:, :, 0:2], v[:, :, 2:4])
                        nc.gpsimd.tensor_add(dst[:, ft, :],
                                             htmp[:, ft, :Sp, 0],
                                             htmp[:, ft, :Sp, 1])

            def emit_fprod(f_sb, ch_c, dst, Sp):
                """dst[P,2,Sp] = sum_children sigmoid(f) * child_c."""
                with nc.allow_low_precision(reason="fp16 by design"):
                    nc.vector.tensor_tensor(f_sb[:], f_sb[:], ch_c,
                                            AluOpType.mult)
                    for ft in range(2):
                        nc.vector.tensor_reduce(
                            dst[:, ft, :],
                            f_sb[:, ft, :].rearrange("p (n b) -> p n b", b=BR),
                            mybir.AxisListType.X, AluOpType.add)

            # ---------------- leaf phase, pipelined with level-7 f ----------
            leaf_hc = [None] * NG

            def emit_leaf(g):
                if g + 1 < NG:
                    xt_g.append(load_x(8, (g + 1) * GL, GL, tag="xleaf"))
                xt = xt_g[g]
                it, ut, ot = gates_block(xt, GL)
                h8 = leafg.tile([P, 2, GL], F16, tag="h8", bufs=2, name="h8")
                c8 = leafg.tile([P, 2, GL], F16, tag="c8", bufs=2, name="c8")
                combine(it, ut, ot, c8[:], h8[:])
                emit_hsum(h8[:], hs7[:, :, g * 512:(g + 1) * 512], 512)
                leaf_hc[g] = (h8, c8)

            def emit_f7(g):
                h8, c8 = leaf_hc[g]
                f_sb = work.tile([P, 2, GL], F16, tag="f7", bufs=2, name="f7")
                for ft in range(2):
                    ps = f_psum(h8[:], x7[:, :, g * 512:(g + 1) * 512], 512, ft)
                    nc.scalar.activation(f_sb[:, ft, :], ps[:], AFT.Sigmoid,
                                         bias=bt[:, 6 + ft:7 + ft])
                emit_fprod(f_sb, c8[:], fcs7[:, :, g * 512:(g + 1) * 512], 512)

            emit_leaf(0)
            emit_leaf(1)
            emit_f7(0)
            emit_leaf(2)
            emit_f7(1)
            emit_leaf(3)
            emit_f7(2)

            # ---------------- level 7 iou in halves, outputs streamed out --
            HL = NLOC[7] // 2   # 1024

            def emit_iou7(h):
                s = slice(h * HL, (h + 1) * HL)
                it, ut, ot = gates_block(x7[:, :, s], HL, hs=hs7[:, :, s])
                combine(it, ut, ot, c7[:, :, s], h7[:, :, s],
                        fcs=fcs7[:, :, s])
                nc.sync.dma_start(out_h[:, :, s].rearrange("a p n -> p a n"),
                                  h7[:, :, s])
                nc.sync.dma_start(out_c[:, :, s].rearrange("a p n -> p a n"),
                                  c7[:, :, s])

            emit_iou7(0)
            emit_f7(3)
            emit_iou7(1)

    nc.compile()
    return nc


def level_offs():
    return [(BR ** l - 1) // (BR - 1) for l in range(D + 1)]


def shard_inputs(x, W_iou_x, b_iou_x, W_iou_h, b_iou_h, W_fx, b_fx, W_fh, b_fh,
                 *_ignored):
    offs = level_offs()
    wx_cat = np.concatenate([W_iou_x, W_fx], axis=0)
    wh_cat = np.concatenate([W_iou_h, W_fh], axis=0)
    wx_d = np.ascontiguousarray(wx_cat.T).reshape(2, P, 1024).astype(np.float16)
    wh_d = np.ascontiguousarray(wh_cat.T).reshape(2, P, 1024).astype(np.float16)
    b_iou = (b_iou_x + b_iou_h).reshape(6, P).T
    b_f = (b_fx + b_fh).reshape(2, P).T
    bias = np.ascontiguousarray(
        np.concatenate([b_iou, b_f], axis=1)).astype(np.float32)
    in_maps = []
    for k in range(NCORES):
        rows = []
        for l in range(CUT, D + 1):
            n = NLOC[l]
            rows.append(x[offs[l] + k * n: offs[l] + (k + 1) * n])
        xl = np.concatenate(rows, axis=0)
        xTk = np.ascontiguousarray(xl.T).reshape(2, P, -1).astype(np.float16)
        in_maps.append({"xT": xTk, "wx": wx_d, "wh": wh_d, "bias": bias})
    return in_maps


def finish_host(results, x, W_iou_x, b_iou_x, W_iou_h, b_iou_h,
                W_fx, b_fx, W_fh, b_fh, *_ignored):
    ncut = BR ** CUT
    npc = ncut // NCORES
    Hc = np.empty((ncut, 256), np.float32)
    Cc = np.empty((ncut, 256), np.float32)
    for k in range(NCORES):
        oh = results[k]["out_h"].astype(np.float32).reshape(256, npc)
        oc = results[k]["out_c"].astype(np.float32).reshape(256, npc)
        Hc[k * npc:(k + 1) * npc] = oh.T
        Cc[k * npc:(k + 1) * npc] = oc.T
    sig = lambda v: 1.0 / (1.0 + np.exp(-v))
    h_next, c_next = Hc, Cc
    for l in range(CUT - 1, -1, -1):
        n, off = BR ** l, (BR ** l - 1) // 3
        xl = x[off:off + n]
        child_h = h_next.reshape(n, BR, 256)
        child_c = c_next.reshape(n, BR, 256)
        chs = child_h.sum(axis=1)
        iou = xl @ W_iou_x.T + b_iou_x + chs @ W_iou_h.T + b_iou_h
        i, o, u = np.split(iou, 3, axis=1)
        i, o, u = sig(i), sig(o), np.tanh(u)
        f = sig(child_h @ W_fh.T + b_fh + (xl @ W_fx.T + b_fx)[:, None, :])
        c = i * u + (f * child_c).sum(axis=1)
        h = o * np.tanh(c)
        h_next, c_next = h, c
    return c_next.astype(np.float32), h_next.astype(np.float32)


# ---------------- public API ----------------

_D = D
_CUT = CUT
_CACHE = {}


def _get_program():
    if "nc" not in _CACHE:
        _CACHE["nc"] = build_program()
    return _CACHE["nc"]


def kernel(x, W_iou_x, b_iou_x, W_iou_h, b_iou_h, W_fx, b_fx, W_fh, b_fh):
    from concourse import bass_utils
    x = np.asarray(x, dtype=np.float32)
    args = [np.asarray(a, dtype=np.float32) for a in
            (W_iou_x, b_iou_x, W_iou_h, b_iou_h, W_fx, b_fx, W_fh, b_fh)]
    nc = _get_program()
    in_maps = shard_inputs(x, *args)
    res = bass_utils.run_bass_kernel_spmd(nc, in_maps,
                                          core_ids=list(range(NCORES)))
    c, h = finish_host(res.results, x, *args)
    return c, h


# revision 13
# speedup vs baseline: 1.3566x; 1.0150x over previous
"""Child-Sum Tree-LSTM (reference.py nn_ChildSumTreeLSTM) on 8 Trainium2
NeuronCores via Bass/Tile, SPMD.

Strategy: everything transposed (features on SBUF partitions, nodes on the
free dimension). Each core owns a contiguous slice of levels 7..8; since
children of a node are contiguous, the leaves->level-7 recursion is fully
core-local (no collectives). Levels 6..0 (5461 nodes) are finished on the
host in numpy during the gather step.

The activation engine is the binding resource (1 elem/cycle/lane plus a
~352-cycle fixed cost per ACTIVATE), so gates are produced from [P, 2048]
four-bank PSUM tiles and activated in N=2048 calls (N=4096 for the tanh(c)
that needs no bias). fp16 replaces bf16 at identical engine throughput for
~8x less quantization noise. Leaf groups are software-pipelined one group
ahead of the level-7 forget-gate matmuls so the TensorEngine never stalls
on the activation chain it just fed (keeps the PE HAM clock at 2.4 GHz).
Level-7 iou runs in halves with per-half output DMA so the final DMA
overlaps compute. Weights/x for later phases load on the Vector engine's
DMA queue in parallel with the Sync queue's leaf x stream.
"""
import sys
sys.path.insert(0, '/opt/trn_rl_repo')
import numpy as np
import concourse.bacc as bacc
import concourse.mybir as mybir
from concourse.tile import TileContext
from concourse.alu_op_type import AluOpType

F32 = mybir.dt.float32
F16 = mybir.dt.float16
AFT = mybir.ActivationFunctionType
P = 128
NCORES = 8
BR = 4
D = 8
CUT = 7

NLOC = {7: 2048, 8: 8192}
LOFF = {7: 0, 8: 2048}
TOTAL_ROWS = 10240
GL = 2048                     # leaf group width (nodes)
NG = NLOC[8] // GL            # 4 leaf groups

# gate emission order: (wx block index, act fn, name); wx free layout is
# [i(256) | o(256) | u(256) | f(256)], bias cols [i0,i1,o0,o1,u0,u1,f0,f1]
GATES = ((0, AFT.Sigmoid, "i"), (2, AFT.Tanh, "u"), (1, AFT.Sigmoid, "o"))


def build_program():
    nc = bacc.Bacc("TRN2", target_bir_lowering=False, debug=False,
                   num_devices=NCORES)
    xT = nc.dram_tensor("xT", [2, P, TOTAL_ROWS], F16, kind="ExternalInput")
    wx = nc.dram_tensor("wx", [2, P, 1024], F16, kind="ExternalInput")
    wh = nc.dram_tensor("wh", [2, P, 1024], F16, kind="ExternalInput")
    bias = nc.dram_tensor("bias", [P, 8], F32, kind="ExternalInput")
    out_h = nc.dram_tensor("out_h", [2, P, NLOC[7]], F16, kind="ExternalOutput")
    out_c = nc.dram_tensor("out_c", [2, P, NLOC[7]], F16, kind="ExternalOutput")

    with TileContext(nc) as tc:
        with tc.tile_pool(name="const", bufs=1) as constp, \
             tc.tile_pool(name="xin", bufs=2) as xin, \
             tc.tile_pool(name="state", bufs=1) as statep, \
             tc.tile_pool(name="leafg", bufs=2) as leafg, \
             tc.tile_pool(name="work", bufs=2) as work, \
             tc.tile_pool(name="psum", bufs=2, space="PSUM") as psum:

            wxt = constp.tile([P, 2, 1024], F16)
            bt = constp.tile([P, 8], F32)
            wht = constp.tile([P, 2, 1024], F16)
            nc.sync.dma_start(wxt[:], wx[:].rearrange("a p n -> p a n"))
            nc.sync.dma_start(bt[:], bias[:])

            def load_x(l, c0, S, tag, bufs=2, split=1):
                t = xin.tile([P, 2, S], F16, tag=tag, bufs=bufs, name=tag)
                w = S // split
                for j in range(split):
                    lo = LOFF[l] + c0 + j * w
                    nc.sync.dma_start(
                        t[:, :, j * w:(j + 1) * w],
                        xT[:, :, lo:lo + w].rearrange("a p n -> p a n"))
                return t

            # group 0's x is split into 512-column pieces so the first
            # matmul starts as soon as wx + one piece have landed; x7/wht
            # transfers queue up behind it instead of competing for HBM
            nc.scalar.dma_start(wht[:], wh[:].rearrange("a p n -> p a n"))
            xt_g = [load_x(8, 0, GL, tag="xleaf", bufs=2, split=4)]
            x7 = load_x(7, 0, NLOC[7], tag="x7", bufs=1)

            # persistent level-7 state
            hs7 = statep.tile([P, 2, NLOC[7]], F16, name="hs7")
            fcs7 = statep.tile([P, 2, NLOC[7]], F16, name="fcs7")
            h7 = statep.tile([P, 2, NLOC[7]], F16, name="h7")
            c7 = statep.tile([P, 2, NLOC[7]], F16, name="c7")

            def fill_iou(ps, xt, S, gi, ft, hs=None):
                """Fill [P, S] psum AP for gate-block gi, feature-tile ft."""
                sl = slice((gi * 2 + ft) * P, (gi * 2 + ft + 1) * P)
                for q in range(0, S, 512):
                    w = min(512, S - q)
                    dst = ps[:, q:q + w]
                    nc.tensor.matmul(dst, wxt[:, 0, sl], xt[:, 0, q:q + w],
                                     start=True, stop=False)
                    nc.tensor.matmul(dst, wxt[:, 1, sl], xt[:, 1, q:q + w],
                                     start=False, stop=hs is None)
                    if hs is not None:
                        nc.tensor.matmul(dst, wht[:, 0, sl], hs[:, 0, q:q + w],
                                         start=False, stop=False)
                        nc.tensor.matmul(dst, wht[:, 1, sl], hs[:, 1, q:q + w],
                                         start=False, stop=True)

            def f_psum(ch_h, xp, S, ft):
                """[P, 4*S] forget-gate pre-activation psum for S parents:
                W_fh @ child_h + (W_fx @ x_parent) broadcast over children."""
                nch = BR * S
                ps = psum.tile([P, 2048], F32, tag="PS", bufs=2, name="psf")
                sl = slice(768 + ft * P, 768 + (ft + 1) * P)
                for q in range(0, nch, 512):
                    w = min(512, nch - q)
                    dst = ps[:, q:q + w]
                    nc.tensor.matmul(dst, wht[:, 0, sl], ch_h[:, 0, q:q + w],
                                     start=True, stop=False)
                    nc.tensor.matmul(dst, wht[:, 1, sl], ch_h[:, 1, q:q + w],
                                     start=False, stop=False)
                    plo, pw = q // BR, w // BR
                    for kt in range(2):
                        rhs = xp[:, kt, plo:plo + pw] \
                            .rearrange("p (n b) -> p n b", b=1) \
                            .broadcast_to([P, pw, BR])
                        nc.tensor.matmul(
                            dst.rearrange("p (n b) -> p n b", b=BR),
                            wxt[:, kt, sl], rhs, start=False, stop=(kt == 1))
                return ps

            def gates_block(xt, S, hs=None):
                """iou gates for S nodes -> (it, ut, ot) [P, 2, S] fp16.
                Gate tiles are shared across phases (bufs=1): the DVE combine
                consumes them long before the ACT queue wraps around.
                For S<=1024, two gate psums are carved from each 4-bank tile
                so the PE gets twice the runway ahead of the ACT drain."""
                tiles = {}
                idx, ps_tile = 0, [None]
                for gi, fn, nm in GATES:
                    gt = work.tile([P, 2, 2048], F16, tag="g" + nm, bufs=1,
                                   name="g" + nm)
                    for ft in range(2):
                        if S <= 1024:
                            if idx % 2 == 0:
                                ps_tile[0] = psum.tile([P, 2048], F32,
                                                       tag="PS", bufs=2,
                                                       name="ps")
                            ps = ps_tile[0][:, (idx % 2) * S:(idx % 2 + 1) * S]
                        else:
                            ps = psum.tile([P, 2048], F32, tag="PS", bufs=2,
                                           name="ps")[:, :S]
                        fill_iou(ps, xt, S, gi, ft, hs)
                        nc.scalar.activation(gt[:, ft, :S], ps, fn,
                                             bias=bt[:, gi * 2 + ft:gi * 2 + ft + 1])
                        idx += 1
                    tiles[nm] = gt[:, :, :S]
                return tiles["i"], tiles["u"], tiles["o"]

            def combine(it, ut, ot, c_dst, h_dst, fcs=None):
                """c = i*u (+ fcs); h = o*tanh(c). tanh reuses ut storage."""
                with nc.allow_low_precision(reason="fp16 by design"):
                    nc.vector.tensor_tensor(c_dst, it, ut, AluOpType.mult)
                    if fcs is not None:
                        nc.vector.tensor_tensor(c_dst, c_dst, fcs, AluOpType.add)
                    nc.scalar.activation(ut, c_dst, AFT.Tanh)
                    nc.vector.tensor_tensor(h_dst, ot, ut, AluOpType.mult)

            def emit_hsum(ch_h, dst, Sp):
                """Sum 4-child groups of ch_h [P,2,4*Sp] into dst [P,2,Sp]."""
                with nc.allow_low_precision(reason="fp16 by design"):
                    htmp = work.tile([P, 2, 512, 2], F16, tag="htmp", bufs=2,
                                     name="htmp")
                    for ft in range(2):
                        v = ch_h[:, ft, :].rearrange("p (n b) -> p n b", b=BR)
                        nc.gpsimd.tensor_add(htmp[:, ft, :Sp, :],
                                             v[:, :, 0:2], v[:, :, 2:4])
                        nc.gpsimd.tensor_add(dst[:, ft, :],
                                             htmp[:, ft, :Sp, 0],
                                             htmp[:, ft, :Sp, 1])

            def emit_fprod(f_sb, ch_c, dst, Sp):
                """dst[P,2,Sp] = sum_children sigmoid(f) * child_c."""
                with nc.allow_low_precision(reason="fp16 by design"):
                    nc.vector.tensor_tensor(f_sb[:], f_sb[:], ch_c,
                                            AluOpType.mult)
                    for ft in range(2):
                        nc.vector.tensor_reduce(
                            dst[:, ft, :],
                            f_sb[:, ft, :].rearrange("p (n b) -> p n b", b=BR),
                            mybir.AxisListType.X, AluOpType.add)

            # ---------------- leaf phase, pipelined with level-7 f ----------
            leaf_hc = [None] * NG

            def emit_leaf(g):
                if g + 1 < NG:
                    xt_g.append(load_x(8, (g + 1) * GL, GL, tag="xleaf"))
                xt = xt_g[g]
                it, ut, ot = gates_block(xt, GL)
                h8 = leafg.tile([P, 2, GL], F16, tag="h8", bufs=2, name="h8")
                c8 = leafg.tile([P, 2, GL], F16, tag="c8", bufs=2, name="c8")
                combine(it, ut, ot, c8[:], h8[:])
                emit_hsum(h8[:], hs7[:, :, g * 512:(g + 1) * 512], 512)
                leaf_hc[g] = (h8, c8)

            def emit_f7(g):
                h8, c8 = leaf_hc[g]
                f_sb = work.tile([P, 2, GL], F16, tag="f7", bufs=2, name="f7")
                for ft in range(2):
                    ps = f_psum(h8[:], x7[:, :, g * 512:(g + 1) * 512], 512, ft)
                    nc.scalar.activation(f_sb[:, ft, :], ps[:], AFT.Sigmoid,
                                         bias=bt[:, 6 + ft:7 + ft])
                emit_fprod(f_sb, c8[:], fcs7[:, :, g * 512:(g + 1) * 512], 512)

            emit_leaf(0)
            emit_leaf(1)
            emit_f7(0)
            emit_leaf(2)
            emit_f7(1)
            emit_leaf(3)
            emit_f7(2)

            # ---------------- level 7 iou in halves, outputs streamed out --
            HL = NLOC[7] // 2   # 1024

            def emit_iou7(h, sub=1):
                s = slice(h * HL, (h + 1) * HL)
                it, ut, ot = gates_block(x7[:, :, s], HL, hs=hs7[:, :, s])
                w = HL // sub
                for j in range(sub):
                    t = slice(h * HL + j * w, h * HL + (j + 1) * w)
                    jw = slice(j * w, (j + 1) * w)
                    combine(it[:, :, jw], ut[:, :, jw], ot[:, :, jw],
                            c7[:, :, t], h7[:, :, t], fcs=fcs7[:, :, t])
                    nc.sync.dma_start(
                        out_h[:, :, t].rearrange("a p n -> p a n"),
                        h7[:, :, t])
                    nc.sync.dma_start(
                        out_c[:, :, t].rearrange("a p n -> p a n"),
                        c7[:, :, t])

            emit_iou7(0)
            emit_f7(3)
            emit_iou7(1, sub=2)

    nc.compile()
    return nc


def level_offs():
    return [(BR ** l - 1) // (BR - 1) for l in range(D + 1)]


def shard_inputs(x, W_iou_x, b_iou_x, W_iou_h, b_iou_h, W_fx, b_fx, W_fh, b_fh,
                 *_ignored):
    offs = level_offs()
    wx_cat = np.concatenate([W_iou_x, W_fx], axis=0)
    wh_cat = np.concatenate([W_iou_h, W_fh], axis=0)
    wx_d = np.ascontiguousarray(wx_cat.T).reshape(2, P, 1024).astype(np.float16)
    wh_d = np.ascontiguousarray(wh_cat.T).reshape(2, P, 1024).astype(np.float16)
    b_iou = (b_iou_x + b_iou_h).reshape(6, P).T
    b_f = (b_fx + b_fh).reshape(2, P).T
    bias = np.ascontiguousarray(
        np.concatenate([b_iou, b_f], axis=1)).astype(np.float32)
    in_maps = []
    for k in range(NCORES):
        rows = []
        for l in range(CUT, D + 1):
            n = NLOC[l]
            rows.append(x[offs[l] + k * n: offs[l] + (k + 1) * n])
        xl = np.concatenate(rows, axis=0)
        xTk = np.ascontiguousarray(xl.T).reshape(2, P, -1).astype(np.float16)
        in_maps.append({"xT": xTk, "wx": wx_d, "wh": wh_d, "bias": bias})
    return in_maps


def finish_host(results, x, W_iou_x, b_iou_x, W_iou_h, b_iou_h,
                W_fx, b_fx, W_fh, b_fh, *_ignored):
    ncut = BR ** CUT
    npc = ncut // NCORES
    Hc = np.empty((ncut, 256), np.float32)
    Cc = np.empty((ncut, 256), np.float32)
    for k in range(NCORES):
        oh = results[k]["out_h"].astype(np.float32).reshape(256, npc)
        oc = results[k]["out_c"].astype(np.float32).reshape(256, npc)
        Hc[k * npc:(k + 1) * npc] = oh.T
        Cc[k * npc:(k + 1) * npc] = oc.T
    sig = lambda v: 1.0 / (1.0 + np.exp(-v))
    h_next, c_next = Hc, Cc
    for l in range(CUT - 1, -1, -1):
        n, off = BR ** l, (BR ** l - 1) // 3
        xl = x[off:off + n]
        child_h = h_next.reshape(n, BR, 256)
        child_c = c_next.reshape(n, BR, 256)
        chs = child_h.sum(axis=1)
        iou = xl @ W_iou_x.T + b_iou_x + chs @ W_iou_h.T + b_iou_h
        i, o, u = np.split(iou, 3, axis=1)
        i, o, u = sig(i), sig(o), np.tanh(u)
        f = sig(child_h @ W_fh.T + b_fh + (xl @ W_fx.T + b_fx)[:, None, :])
        c = i * u + (f * child_c).sum(axis=1)
        h = o * np.tanh(c)
        h_next, c_next = h, c
    return c_next.astype(np.float32), h_next.astype(np.float32)


# ---------------- public API ----------------

_D = D
_CUT = CUT
_CACHE = {}


def _get_program():
    if "nc" not in _CACHE:
        _CACHE["nc"] = build_program()
    return _CACHE["nc"]


def kernel(x, W_iou_x, b_iou_x, W_iou_h, b_iou_h, W_fx, b_fx, W_fh, b_fh):
    from concourse import bass_utils
    x = np.asarray(x, dtype=np.float32)
    args = [np.asarray(a, dtype=np.float32) for a in
            (W_iou_x, b_iou_x, W_iou_h, b_iou_h, W_fx, b_fx, W_fh, b_fh)]
    nc = _get_program()
    in_maps = shard_inputs(x, *args)
    res = bass_utils.run_bass_kernel_spmd(nc, in_maps,
                                          core_ids=list(range(NCORES)))
    c, h = finish_host(res.results, x, *args)
    return c, h
